# revision 37
# baseline (speedup 1.0000x reference)
"""MultiHeadSimilarity kernel for 8 Trainium2 NeuronCores.

Reference computation (per batch b):
    Q = wq @ x[b];  K = wk @ y[b]                       (channel-mixing matmuls)
    per head h (d=64):  A = relu(Qh^T Kh) * scale, masked by xy_mask
    C = A @ Kh^T, normalized per-row by 1/max(sum(mask, y), 1)
    out = wo @ (0.5 * (Q + C))

Sharding: data-parallel over batch; 16 batches / 8 cores = 2 per core.
Weights replicated. No cross-core communication.

Device algorithm (per core, fp16 compute with fp32 PSUM accumulation):
  - The mask row-normalization (1/(8*max(nel,1)), which also folds the
    1/sqrt(d) attention scale) is computed on the HOST and pre-multiplied
    into the transposed mask -> MTI.  The device then computes
    Am = relu(A) * MTI and C comes out of the PE pre-normalized; E is a
    single add of C (PSUM) and Q.  This removes the ones^T@mask row-count
    matmuls, the reciprocal chain and the per-tile E multiply.
  - KT (the K transpose needed as the C-contraction stationary) is made by
    PE transpose instructions from K (4 per y-tile, via an identity), not by
    a second full projection: 4096 instead of 16384 PE columns per batch.
  - A is computed transposed (y on partitions) per head; relu+mask are
    balanced across three engines: DVE scalar_tensor_tensor, or ScalarE
    relu + multiply on DVE or GpSimd.  0.5 is folded into woT on the host.
"""
import sys

if "/opt/trn_rl_repo" not in sys.path:
    sys.path.insert(0, "/opt/trn_rl_repo")

import numpy as np

import concourse.tile as tile
from concourse import bacc, masks, mybir
from concourse.bass_utils import run_bass_kernel_spmd

F16 = mybir.dt.float16
F32 = mybir.dt.float32
AL = mybir.AluOpType
RELU = mybir.ActivationFunctionType.Relu

N_CORES = 8
B, U, LX, LY, H, D = 16, 512, 1024, 1024, 8, 64
BPC = B // N_CORES          # batches per core
KB = U // 128               # 4  k-tiles over channels
HP = H // 2                 # 4  head pairs
YT = LY // 128              # 8  y tiles
XH = LX // 512              # 2  x halves
INV_SCALE = float(D) ** 0.5  # 8.0; attention scale = 1/8 (folded into MTI)

TRACE = False
_CACHE = {}


class Balance:
    """Greedy static load balancer between DVE and ACT.

    GpSimd is deliberately NOT used for element-wise work: its throughput is
    ~3x worse than modeled, its queue drowns in semaphore updates, and the
    resulting Am-supply stalls make the PE idle long enough for the HAM
    activity monitor to re-throttle the PE clock to 1.2 GHz (measured: 52%
    of the runtime at K=4/8, a net regression).
    """

    # measured per-op costs (ns) as functions of free-dim length
    @staticmethod
    def dve_psum(fd):          # psum-f32-source op (stt / cast / copy): 1x mode
        return (fd + 130) / 0.96

    @staticmethod
    def dve_f16(fd):           # f16 x f16 -> f16 tensor_tensor: 2x mode
        return (fd / 2 + 130) / 0.96

    @staticmethod
    def act_op(fd):            # scalar-engine activation / copy
        return (fd + 290) / 1.2

    GPS_TT = 2400.0            # gpsimd f16 tensor_tensor, per (128,2,512) tile

    def __init__(self, nc):
        self.nc = nc
        self.t = {"v": 0.0, "s": 0.0, "g": 0.0}

    def copy(self, dst, src, fd):
        """psum -> sbuf copy, DVE or ACT."""
        dve = self.dve_psum(fd)
        act = self.act_op(fd)
        if self.t["v"] + dve <= self.t["s"] + act:
            self.t["v"] += dve
            self.nc.vector.tensor_copy(dst, src)
        else:
            self.t["s"] += act
            self.nc.scalar.copy(dst, src)

    def relu_mask_pair(self, out, a_ps, mtf_b, mti_2d, tmp_pool, name,
                       lat_ok=False):
        """out[(128,2,512) f16] = relu(a_ps) * MTI broadcast over head dim.

        route 1: one fused DVE scalar_tensor_tensor at FD=1024;
        route 2: ScalarE relu at FD=1024 + DVE f16 2x multiply;
        route 3 (lat_ok only): ScalarE relu + GpSimd multiply.  GpSimd is
        ~3.5x slower than DVE and its op latency is ~2.5us, so it only gets
        tiles whose consumer is more than a block away (the fill phase,
        where DVE/ACT capacity is what limits Am inventory build-up) and
        only a handful per batch (a saturated Pool queue stalls everything).
        """
        stt = self.dve_psum(1024)
        act = self.act_op(1024)
        mul_v = self.dve_f16(1024)
        m1 = max(self.t["v"] + stt, self.t["s"], self.t["g"])
        m2 = max(self.t["v"] + mul_v, self.t["s"] + act, self.t["g"])
        # measured: even ~6 GpSimd multiplies per batch slow the whole run
        # (Pool-queue semaphore overhead + SBUF-port contention with DVE);
        # route 3 kept for reference but disabled
        gps_ok = False and lat_ok and self.t["g"] + self.GPS_TT <= min(
            self.t["v"], self.t["s"])
        m3 = max(self.t["v"], self.t["s"] + act, self.t["g"] + self.GPS_TT) \
            if gps_ok else float("inf")
        best = min(m1, m2, m3)
        if best == m1:
            self.t["v"] += stt
            self.nc.vector.scalar_tensor_tensor(out[:], a_ps[:], 0.0, mtf_b,
                                                AL.max, AL.mult)
        else:
            at = tmp_pool.tile([128, 2, 512], F16, tag="at", name=name)
            self.t["s"] += act
            self.nc.scalar.activation(at[:], a_ps[:], RELU)
            if best == m2:
                self.t["v"] += mul_v
                self.nc.vector.tensor_tensor(out[:], at[:], mtf_b, AL.mult)
            else:
                self.t["g"] += self.GPS_TT
                for j in range(2):
                    self.nc.gpsimd.tensor_tensor(out[:, j, :], at[:, j, :],
                                                 mti_2d, AL.mult)


def _build():
    nc = bacc.Bacc("TRN2", target_bir_lowering=False, debug=False,
                   num_devices=N_CORES)
    x_e = nc.dram_tensor("x", [BPC, U, LX], F16, kind="ExternalInput")
    y_e = nc.dram_tensor("y", [BPC, U, LY], F16, kind="ExternalInput")
    mt_e = nc.dram_tensor("mt", [BPC, LY, LX], F16, kind="ExternalInput")
    w_all_e = nc.dram_tensor("w_all", [3, U, U], F16, kind="ExternalInput")
    o_e = nc.dram_tensor("o", [BPC, U, LX], F32, kind="ExternalOutput")

    with tile.TileContext(nc) as tc:
        _emit(nc, tc, x_e, y_e, mt_e, w_all_e, o_e)
    nc.compile()
    return nc


def _emit(nc, tc, x_e, y_e, mt_e, w_all_e, o_e):
    import contextlib
    bal = Balance(nc)
    ctx = contextlib.ExitStack()
    with ctx:
        wp = ctx.enter_context(tc.tile_pool(name="wp", bufs=1))
        io = ctx.enter_context(tc.tile_pool(name="io", bufs=2))
        pr = ctx.enter_context(tc.tile_pool(name="pr", bufs=2))
        amp = ctx.enter_context(tc.tile_pool(name="amp", bufs=4))
        osp = ctx.enter_context(tc.tile_pool(name="osp", bufs=4))
        pa = ctx.enter_context(tc.tile_pool(name="pa", bufs=3, space="PSUM"))
        pc = ctx.enter_context(tc.tile_pool(name="pc", bufs=2, space="PSUM"))

        # weights, loaded once (per-k so the first projection can start early)
        WQT = wp.tile([128, KB, U], F16, tag="wqt")
        WKT = wp.tile([128, KB, U], F16, tag="wkt")
        WOT = wp.tile([128, KB, U], F16, tag="wot")
        for wi, w_t in enumerate((WQT, WKT)):
            for k in range(KB):
                nc.scalar.dma_start(
                    w_t[:, k, :], w_all_e.ap()[wi, k * 128:(k + 1) * 128, :])
        ident = wp.tile([128, 128], F16, tag="ident")
        masks.make_identity(nc, ident[:])

        def issue_loads(b):
            """DMA-issue one batch's inputs; returns the SBUF tiles.

            X fully precedes Y precedes MTI (the order the PE consumes
            them), and consecutive k/y tiles alternate between the sync and
            gpsimd queues so each tensor streams at the combined bandwidth
            instead of serializing behind one ring.
            """
            X = io.tile([128, KB, LX], F16, tag="x", name=f"x{b}")
            Y = io.tile([128, KB, LY], F16, tag="y", name=f"y{b}")
            for src, dst in ((x_e, X), (y_e, Y)):
                for k in range(KB):
                    (nc.sync if k % 2 == 0 else nc.gpsimd).dma_start(
                        dst[:, k, :], src.ap()[b, k * 128:(k + 1) * 128, :])
            MTI = io.tile([128, YT, LX], F16, tag="mti", name=f"mti{b}")
            # batch 0 only: the first two mask tiles ride the scalar queue
            # (free once the wq/wk chunks are in) — the fill's relu+mask ops
            # are otherwise mask-starved while MTI queues behind X and Y.
            # Later batches prefetch during compute, when the scalar queue
            # is busy with relu work.
            if b == 0:
                qs = [nc.scalar, nc.scalar, nc.sync, nc.gpsimd,
                      nc.sync, nc.gpsimd, nc.sync, nc.gpsimd]
            else:
                qs = [nc.sync, nc.gpsimd] * 4
            for t in range(YT):
                qs[t].dma_start(
                    MTI[:, t, :], mt_e.ap()[b, t * 128:(t + 1) * 128, :])
            return X, Y, MTI

        loaded = {0: issue_loads(0)}
        # Warm-up: ~30 back-to-back identity transposes (results discarded).
        # The HAM clock gate keeps the PE at 1.2 GHz until it sees ~3.4us of
        # sustained activity; without this burst the whole DMA-bound fill
        # phase (first ~25us of matmuls) runs at half clock.
        jk = pc.tile([128, 4, 128], F16, tag="c", name="warmup")
        for i in range(30):
            nc.tensor.transpose(jk[:, i % 4, :], ident[:], ident[:])
        # WOT is not needed until the first output projection (~80us in);
        # keeping it out of the startup window frees 0.5MB of HBM bandwidth
        # for the batch-0 activations the PE is waiting on
        for k in range(KB):
            nc.scalar.dma_start(
                WOT[:, k, :], w_all_e.ap()[2, k * 128:(k + 1) * 128, :])

        st = {}   # per-batch state

        def init_batch(b):
            X, Y, MTI = loaded.pop(b)
            st[b] = dict(
                X=X, Y=Y, MTI=MTI,
                Q=pr.tile([128, KB, LX], F16, tag="q", name=f"q{b}"),
                K=pr.tile([128, KB, LY], F16, tag="k", name=f"k{b}"),
                KT=pr.tile([128, YT, U], F16, tag="kt", name=f"kt{b}"),
                E=pr.tile([128, KB, LX], F16, tag="e", name=f"e{b}"),
                ams={}, cs={},
            )

        def emit_proj(b, w_t, sk, dk, m):
            s = st[b]
            src, dst = s[sk], s[dk]
            ps = pa.tile([128, 2, 512], F32, tag="a", name=f"pj{b}_{dk}_{m}")
            for k in range(KB):
                for n in range(XH):
                    nc.tensor.matmul(
                        ps[:, n, :], w_t[:, k, m * 128:(m + 1) * 128],
                        src[:, k, n * 512:(n + 1) * 512],
                        start=(k == 0), stop=(k == KB - 1))
            bal.copy(dst[:, m, :], ps[:], 1024)

        def emit_transpose(b, yt):
            s = st[b]
            ktp = pc.tile([128, 4, 128], F16, tag="c", name=f"ktp{b}_{yt}")
            for k in range(KB):
                nc.tensor.transpose(
                    ktp[:, k, :], s["K"][:, k, yt * 128:(yt + 1) * 128],
                    ident[:])
            bal.copy(s["KT"][:, yt, :], ktp[:], 512)

        def emit_a(b, i, yt, lat_ok=False):
            s = st[b]
            hp, xh = i // 2, i % 2
            xs = slice(xh * 512, (xh + 1) * 512)
            A = pa.tile([128, 2, 512], F32, tag="a", name=f"a_{b}_{i}_{yt}")
            for j in range(2):
                hs = slice(64 * j, 64 * (j + 1))
                nc.tensor.matmul(
                    A[:, j, :], s["K"][hs, hp, yt * 128:(yt + 1) * 128],
                    s["Q"][hs, hp, xs], start=True, stop=True)
            Am = amp.tile([128, 2, 512], F16, tag="am", bufs=18,
                          name=f"am_{b}_{i}_{yt}")
            mtf_b = s["MTI"][:, yt, xs].unsqueeze(1).broadcast_to((128, 2, 512))
            bal.relu_mask_pair(Am, A, mtf_b, s["MTI"][:, yt, xs], amp,
                               f"at_{b}_{i}_{yt}", lat_ok=lat_ok)
            s["ams"][(i, yt)] = Am

        def emit_c(b, i, yt):
            s = st[b]
            hp, xh = i // 2, i % 2
            # both heads accumulate into ONE bank: j0 at partitions 0-63
            # (col group 0), j1 at 64-127 (col group 64).
            if yt == 0:
                s["cs"][i] = pc.tile([128, 512], F32, tag="c", name=f"c_{b}_{i}")
            C = s["cs"][i]
            for j in range(2):
                nc.tensor.matmul(
                    C[64 * j:64 * (j + 1), :],
                    s["KT"][:, yt, hp * 128 + 64 * j: hp * 128 + 64 * (j + 1)],
                    s["ams"][(i, yt)][:, j, :],
                    start=(yt == 0), stop=(yt == YT - 1),
                    skip_group_check=True)
            if yt == YT - 1:
                # E = Q + C (C is pre-normalized via MTI); DVE only
                # (ACT has no tensor-tensor, GpSimd has no PSUM access)
                xs = slice(xh * 512, (xh + 1) * 512)
                nc.vector.tensor_tensor(s["E"][:, hp, xs], C[:],
                                        s["Q"][:, hp, xs], AL.add)
                bal.t["v"] += bal.dve_psum(512)

        def emit_out(b, m):
            # n-outer: each x-half's accumulation group closes after 4
            # matmuls, so its copy + store DMA overlap the other half's
            # matmuls (shortens the kernel tail)
            s = st[b]
            ps = pa.tile([128, 2, 512], F32, tag="a", name=f"po{b}_{m}")
            oS = osp.tile([128, 2, 512], F32, tag="os", name=f"os{b}_{m}")
            for n in range(XH):
                for k in range(KB):
                    nc.tensor.matmul(ps[:, n, :],
                                     WOT[:, k, m * 128:(m + 1) * 128],
                                     s["E"][:, k, n * 512:(n + 1) * 512],
                                     start=(k == 0), stop=(k == KB - 1))
                bal.copy(oS[:, n, :], ps[:, n, :], 512)
                nc.sync.dma_start(
                    o_e.ap()[b, m * 128:(m + 1) * 128, n * 512:(n + 1) * 512],
                    oS[:, n, :])

        def junk(n):
            # identity transposes, discarded: keep the PE "busy" through
            # DMA-bound stretches so the HAM clock gate stays at 2.4 GHz.
            # All junk() calls precede the first emit_transpose in emission
            # order, so the pc-pool slot handoff from jk to the ktp tiles
            # is cleanly ordered.
            for i in range(n):
                nc.tensor.transpose(jk[:, i % 4, :], ident[:], ident[:])

        def fill(b, warm):
            """Projections + a-blocks 0/1 + KT transposes for batch b.
            warm: sprinkle junk transposes (batch 0 only, DMA-bound)."""
            J = (lambda n: junk(n)) if warm else (lambda n: None)
            emit_proj(b, WQT, "X", "Q", 0)
            J(5)
            emit_proj(b, WKT, "Y", "K", 0)
            J(5)
            emit_a(b, 0, 0, True); emit_a(b, 0, 1, True)
            emit_proj(b, WKT, "Y", "K", 1)
            J(4)
            emit_a(b, 0, 2, True); emit_a(b, 0, 3, True)
            emit_proj(b, WKT, "Y", "K", 2)
            J(4)
            emit_a(b, 0, 4, True); emit_a(b, 0, 5, True)
            emit_proj(b, WKT, "Y", "K", 3)
            J(4)
            emit_a(b, 0, 6, True); emit_a(b, 0, 7, True)
            emit_proj(b, WQT, "X", "Q", 1)
            J(4)
            for yt in range(YT):
                emit_a(b, 1, yt, lat_ok=True)
                emit_transpose(b, yt)
            if b + 1 < BPC:
                loaded[b + 1] = issue_loads(b + 1)

        def steady(b):
            # A(i) and C(i-2) interleaved per y-tile: the relu+mask demand
            # on DVE/ACT stays smooth (1 op per ~1.4us of PE work) and each
            # C matmul trails its Am op by two full blocks
            for i in range(2, 2 * HP):
                if i == 2:
                    emit_proj(b, WQT, "X", "Q", 2)
                if i == 4:
                    emit_proj(b, WQT, "X", "Q", 3)
                for yt in range(YT):
                    emit_a(b, i, yt)
                    emit_c(b, i - 2, yt)

        # ---- the woven whole-kernel schedule ----
        # Per batch: fill (projections + a-blocks 0,1 + transposes), steady
        # (a2..a7 against c0..c5), then the c6/c7 drain and the output
        # projection are INTERLEAVED with the next batch's fill so the
        # PSUM->SBUF copy and relu+mask load on DVE/ACT stays smooth across
        # the batch boundary (a solid block of those stalls the PE long
        # enough for the HAM clock gate to re-throttle it to 1.2 GHz).
        init_batch(0)
        fill(0, warm=True)
        steady(0)
        for b in range(1, BPC):
            p = b - 1
            init_batch(b)
            for yt in range(YT):
                emit_c(p, 2 * HP - 2, yt)
                if yt == 1:
                    emit_proj(b, WQT, "X", "Q", 0)
                if yt == 5:
                    emit_proj(b, WKT, "Y", "K", 0)
            for yt in range(YT):
                emit_c(p, 2 * HP - 1, yt)
                if yt < 4:
                    emit_a(b, 0, yt)
            emit_out(p, 0)
            emit_a(b, 0, 4, True); emit_a(b, 0, 5, True)
            emit_proj(b, WKT, "Y", "K", 1)
            emit_out(p, 1)
            emit_a(b, 0, 6, True); emit_a(b, 0, 7, True)
            emit_proj(b, WKT, "Y", "K", 2)
            emit_out(p, 2)
            emit_a(b, 1, 0); emit_a(b, 1, 1)
            emit_proj(b, WKT, "Y", "K", 3)
            emit_out(p, 3)
            emit_proj(b, WQT, "X", "Q", 1)
            for yt in range(YT):
                if yt >= 2:
                    emit_a(b, 1, yt)
                emit_transpose(b, yt)
            if b + 1 < BPC:
                loaded[b + 1] = issue_loads(b + 1)
            steady(b)
        # ---- last-batch drain + output projection, k-pipelined ----
        # E's k-blocks 0..2 (head-pairs 0..2) are complete before the c6/c7
        # drain, so the out-projection's k=0..2 accumulation matmuls weave
        # into the drain; only the k=3 matmuls, copies and store DMAs remain
        # after the final E-add, cutting the kernel tail from ~13.6us to ~4.
        last = BPC - 1
        ops = {}

        def out_partial(m, k):
            if m not in ops:
                ops[m] = (pa.tile([128, 2, 512], F32, tag="a",
                                  name=f"po{last}_{m}"),
                          osp.tile([128, 2, 512], F32, tag="os",
                                   name=f"os{last}_{m}"))
            ps, _ = ops[m]
            for n in range(XH):
                nc.tensor.matmul(ps[:, n, :],
                                 WOT[:, k, m * 128:(m + 1) * 128],
                                 st[last]["E"][:, k, n * 512:(n + 1) * 512],
                                 start=(k == 0), stop=(k == KB - 1))

        chunks = [(m, k) for m in range(2) for k in range(KB - 1)]
        ci = 0
        for i in (2 * HP - 2, 2 * HP - 1):
            for yt in range(YT):
                emit_c(last, i, yt)
                if yt % 2 == 0 and ci < len(chunks):
                    out_partial(*chunks[ci])
                    ci += 1
        while ci < len(chunks):
            out_partial(*chunks[ci])
            ci += 1
        # m0/m1 close with one k=3 matmul each; m2/m3 run in full while the
        # m0/m1 copies and store DMAs drain on the other engines
        out_partial(0, KB - 1)
        out_partial(1, KB - 1)

        def out_store(m, n, eng=None):
            ps, oS = ops[m]
            if eng is None:
                bal.copy(oS[:, n, :], ps[:, n, :], 512)
            elif eng == "v":
                nc.vector.tensor_copy(oS[:, n, :], ps[:, n, :])
            else:
                nc.scalar.copy(oS[:, n, :], ps[:, n, :])
            nc.sync.dma_start(
                o_e.ap()[last, m * 128:(m + 1) * 128, n * 512:(n + 1) * 512],
                oS[:, n, :])

        out_store(0, 0)
        out_store(0, 1)
        for k in range(KB):
            out_partial(2, k)
        out_store(1, 0)
        out_store(1, 1)
        for k in range(KB):
            out_partial(3, k)
        out_store(2, 0, "v")
        out_store(2, 1, "s")
        out_store(3, 0, "v")
        out_store(3, 1, "s")


def _get_nc():
    if "nc" not in _CACHE:
        _CACHE["nc"] = _build()
    return _CACHE["nc"]


def kernel(x, y, xy_mask, wq, wk, wo):
    nc = _get_nc()
    xf = x.astype(np.float16)
    yf = y.astype(np.float16)
    # fold the attention scale and the per-row 1/nel normalization into the
    # transposed mask on the host: MTI[y, x] = mask[x, y] / (8 * max(nel_x, 1))
    nel = xy_mask.sum(axis=2, dtype=np.float32)           # (B, Lx)
    inv = 1.0 / (INV_SCALE * np.maximum(nel, 1.0))
    mtt = (xy_mask.transpose(0, 2, 1).astype(np.float32)
           * inv[:, None, :]).astype(np.float16)
    mtt = np.ascontiguousarray(mtt)
    w_all = np.stack([wq.T, wk.T, (0.5 * wo).T]).astype(np.float16)
    w_all = np.ascontiguousarray(w_all)
    in_maps = [
        {"x": xf[c * BPC:(c + 1) * BPC], "y": yf[c * BPC:(c + 1) * BPC],
         "mt": mtt[c * BPC:(c + 1) * BPC], "w_all": w_all}
        for c in range(N_CORES)
    ]
    res = run_bass_kernel_spmd(nc, in_maps, list(range(N_CORES)), trace=TRACE)
    if TRACE:
        _CACHE["last_exec_time_ns"] = res.exec_time_ns
        _CACHE["last_profile_json"] = res.profile_json
    return np.concatenate([res.results[c]["o"] for c in range(N_CORES)], axis=0)


# revision 38
# speedup vs baseline: 1.0042x; 1.0042x over previous
"""MultiHeadSimilarity kernel for 8 Trainium2 NeuronCores.

Reference computation (per batch b):
    Q = wq @ x[b];  K = wk @ y[b]                       (channel-mixing matmuls)
    per head h (d=64):  A = relu(Qh^T Kh) * scale, masked by xy_mask
    C = A @ Kh^T, normalized per-row by 1/max(sum(mask, y), 1)
    out = wo @ (0.5 * (Q + C))

Sharding: data-parallel over batch; 16 batches / 8 cores = 2 per core.
Weights replicated. No cross-core communication.

Device algorithm (per core, fp16 compute with fp32 PSUM accumulation):
  - The mask row-normalization (1/(8*max(nel,1)), which also folds the
    1/sqrt(d) attention scale) is computed on the HOST and pre-multiplied
    into the transposed mask -> MTI.  The device then computes
    Am = relu(A) * MTI and C comes out of the PE pre-normalized; E is a
    single add of C (PSUM) and Q.  This removes the ones^T@mask row-count
    matmuls, the reciprocal chain and the per-tile E multiply.
  - KT (the K transpose needed as the C-contraction stationary) is made by
    PE transpose instructions from K (4 per y-tile, via an identity), not by
    a second full projection: 4096 instead of 16384 PE columns per batch.
  - A is computed transposed (y on partitions) per head; relu+mask are
    balanced across three engines: DVE scalar_tensor_tensor, or ScalarE
    relu + multiply on DVE or GpSimd.  0.5 is folded into woT on the host.
"""
import sys

if "/opt/trn_rl_repo" not in sys.path:
    sys.path.insert(0, "/opt/trn_rl_repo")

import numpy as np

import concourse.tile as tile
from concourse import bacc, masks, mybir
from concourse.bass_utils import run_bass_kernel_spmd

F16 = mybir.dt.float16
F32 = mybir.dt.float32
AL = mybir.AluOpType
RELU = mybir.ActivationFunctionType.Relu

N_CORES = 8
B, U, LX, LY, H, D = 16, 512, 1024, 1024, 8, 64
BPC = B // N_CORES          # batches per core
KB = U // 128               # 4  k-tiles over channels
HP = H // 2                 # 4  head pairs
YT = LY // 128              # 8  y tiles
XH = LX // 512              # 2  x halves
INV_SCALE = float(D) ** 0.5  # 8.0; attention scale = 1/8 (folded into MTI)

TRACE = False
_CACHE = {}


class Balance:
    """Greedy static load balancer between DVE and ACT.

    GpSimd is deliberately NOT used for element-wise work: its throughput is
    ~3x worse than modeled, its queue drowns in semaphore updates, and the
    resulting Am-supply stalls make the PE idle long enough for the HAM
    activity monitor to re-throttle the PE clock to 1.2 GHz (measured: 52%
    of the runtime at K=4/8, a net regression).
    """

    # measured per-op costs (ns) as functions of free-dim length
    @staticmethod
    def dve_psum(fd):          # psum-f32-source op (stt / cast / copy): 1x mode
        return (fd + 130) / 0.96

    @staticmethod
    def dve_f16(fd):           # f16 x f16 -> f16 tensor_tensor: 2x mode
        return (fd / 2 + 130) / 0.96

    @staticmethod
    def act_op(fd):            # scalar-engine activation / copy
        return (fd + 290) / 1.2

    GPS_TT = 2400.0            # gpsimd f16 tensor_tensor, per (128,2,512) tile

    def __init__(self, nc):
        self.nc = nc
        self.t = {"v": 0.0, "s": 0.0, "g": 0.0}

    def copy(self, dst, src, fd):
        """psum -> sbuf copy, DVE or ACT."""
        dve = self.dve_psum(fd)
        act = self.act_op(fd)
        if self.t["v"] + dve <= self.t["s"] + act:
            self.t["v"] += dve
            self.nc.vector.tensor_copy(dst, src)
        else:
            self.t["s"] += act
            self.nc.scalar.copy(dst, src)

    def relu_mask_pair(self, out, a_ps, mtf_b, mti_2d, tmp_pool, name,
                       lat_ok=False):
        """out[(128,2,512) f16] = relu(a_ps) * MTI broadcast over head dim.

        route 1: one fused DVE scalar_tensor_tensor at FD=1024;
        route 2: ScalarE relu at FD=1024 + DVE f16 2x multiply;
        route 3 (lat_ok only): ScalarE relu + GpSimd multiply.  GpSimd is
        ~3.5x slower than DVE and its op latency is ~2.5us, so it only gets
        tiles whose consumer is more than a block away (the fill phase,
        where DVE/ACT capacity is what limits Am inventory build-up) and
        only a handful per batch (a saturated Pool queue stalls everything).
        """
        stt = self.dve_psum(1024)
        act = self.act_op(1024)
        mul_v = self.dve_f16(1024)
        m1 = max(self.t["v"] + stt, self.t["s"], self.t["g"])
        m2 = max(self.t["v"] + mul_v, self.t["s"] + act, self.t["g"])
        # measured: even ~6 GpSimd multiplies per batch slow the whole run
        # (Pool-queue semaphore overhead + SBUF-port contention with DVE);
        # route 3 kept for reference but disabled
        gps_ok = False and lat_ok and self.t["g"] + self.GPS_TT <= min(
            self.t["v"], self.t["s"])
        m3 = max(self.t["v"], self.t["s"] + act, self.t["g"] + self.GPS_TT) \
            if gps_ok else float("inf")
        best = min(m1, m2, m3)
        if best == m1:
            self.t["v"] += stt
            self.nc.vector.scalar_tensor_tensor(out[:], a_ps[:], 0.0, mtf_b,
                                                AL.max, AL.mult)
        else:
            at = tmp_pool.tile([128, 2, 512], F16, tag="at", name=name)
            self.t["s"] += act
            self.nc.scalar.activation(at[:], a_ps[:], RELU)
            if best == m2:
                self.t["v"] += mul_v
                self.nc.vector.tensor_tensor(out[:], at[:], mtf_b, AL.mult)
            else:
                self.t["g"] += self.GPS_TT
                for j in range(2):
                    self.nc.gpsimd.tensor_tensor(out[:, j, :], at[:, j, :],
                                                 mti_2d, AL.mult)


def _build():
    nc = bacc.Bacc("TRN2", target_bir_lowering=False, debug=False,
                   num_devices=N_CORES)
    x_e = nc.dram_tensor("x", [BPC, U, LX], F16, kind="ExternalInput")
    y_e = nc.dram_tensor("y", [BPC, U, LY], F16, kind="ExternalInput")
    mt_e = nc.dram_tensor("mt", [BPC, LY, LX], F16, kind="ExternalInput")
    w_all_e = nc.dram_tensor("w_all", [3, U, U], F16, kind="ExternalInput")
    o_e = nc.dram_tensor("o", [BPC, U, LX], F32, kind="ExternalOutput")

    with tile.TileContext(nc) as tc:
        _emit(nc, tc, x_e, y_e, mt_e, w_all_e, o_e)
    nc.compile()
    return nc


def _emit(nc, tc, x_e, y_e, mt_e, w_all_e, o_e):
    import contextlib
    bal = Balance(nc)
    ctx = contextlib.ExitStack()
    with ctx:
        wp = ctx.enter_context(tc.tile_pool(name="wp", bufs=1))
        io = ctx.enter_context(tc.tile_pool(name="io", bufs=2))
        pr = ctx.enter_context(tc.tile_pool(name="pr", bufs=2))
        amp = ctx.enter_context(tc.tile_pool(name="amp", bufs=4))
        osp = ctx.enter_context(tc.tile_pool(name="osp", bufs=4))
        pa = ctx.enter_context(tc.tile_pool(name="pa", bufs=3, space="PSUM"))
        pc = ctx.enter_context(tc.tile_pool(name="pc", bufs=2, space="PSUM"))

        # weights, loaded once (per-k so the first projection can start early)
        WQT = wp.tile([128, KB, U], F16, tag="wqt")
        WKT = wp.tile([128, KB, U], F16, tag="wkt")
        WOT = wp.tile([128, KB, U], F16, tag="wot")
        for wi, w_t in enumerate((WQT, WKT)):
            for k in range(KB):
                nc.scalar.dma_start(
                    w_t[:, k, :], w_all_e.ap()[wi, k * 128:(k + 1) * 128, :])
        ident = wp.tile([128, 128], F16, tag="ident")
        masks.make_identity(nc, ident[:])

        def issue_loads(b):
            """DMA-issue one batch's inputs; returns the SBUF tiles.

            X fully precedes Y precedes MTI (the order the PE consumes
            them), and consecutive k/y tiles alternate between the sync and
            gpsimd queues so each tensor streams at the combined bandwidth
            instead of serializing behind one ring.
            """
            X = io.tile([128, KB, LX], F16, tag="x", name=f"x{b}")
            Y = io.tile([128, KB, LY], F16, tag="y", name=f"y{b}")
            for src, dst in ((x_e, X), (y_e, Y)):
                for k in range(KB):
                    (nc.sync if k % 2 == 0 else nc.gpsimd).dma_start(
                        dst[:, k, :], src.ap()[b, k * 128:(k + 1) * 128, :])
            MTI = io.tile([128, YT, LX], F16, tag="mti", name=f"mti{b}")
            # batch 0 only: the first two mask tiles ride the scalar queue
            # (free once the wq/wk chunks are in) — the fill's relu+mask ops
            # are otherwise mask-starved while MTI queues behind X and Y.
            # Later batches prefetch during compute, when the scalar queue
            # is busy with relu work.
            if b == 0:
                qs = [nc.scalar, nc.scalar, nc.sync, nc.gpsimd,
                      nc.sync, nc.gpsimd, nc.sync, nc.gpsimd]
            else:
                qs = [nc.sync, nc.gpsimd] * 4
            for t in range(YT):
                qs[t].dma_start(
                    MTI[:, t, :], mt_e.ap()[b, t * 128:(t + 1) * 128, :])
            return X, Y, MTI

        loaded = {0: issue_loads(0)}
        # Warm-up: ~30 back-to-back identity transposes (results discarded).
        # The HAM clock gate keeps the PE at 1.2 GHz until it sees ~3.4us of
        # sustained activity; without this burst the whole DMA-bound fill
        # phase (first ~25us of matmuls) runs at half clock.
        jk = pc.tile([128, 4, 128], F16, tag="c", name="warmup")
        for i in range(40):
            nc.tensor.transpose(jk[:, i % 4, :], ident[:], ident[:])
        # WOT is not needed until the first output projection (~80us in);
        # keeping it out of the startup window frees 0.5MB of HBM bandwidth
        # for the batch-0 activations the PE is waiting on
        for k in range(KB):
            nc.scalar.dma_start(
                WOT[:, k, :], w_all_e.ap()[2, k * 128:(k + 1) * 128, :])

        st = {}   # per-batch state

        def init_batch(b):
            X, Y, MTI = loaded.pop(b)
            st[b] = dict(
                X=X, Y=Y, MTI=MTI,
                Q=pr.tile([128, KB, LX], F16, tag="q", name=f"q{b}"),
                K=pr.tile([128, KB, LY], F16, tag="k", name=f"k{b}"),
                KT=pr.tile([128, YT, U], F16, tag="kt", name=f"kt{b}"),
                E=pr.tile([128, KB, LX], F16, tag="e", name=f"e{b}"),
                ams={}, cs={},
            )

        def emit_proj(b, w_t, sk, dk, m):
            s = st[b]
            src, dst = s[sk], s[dk]
            ps = pa.tile([128, 2, 512], F32, tag="a", name=f"pj{b}_{dk}_{m}")
            for k in range(KB):
                for n in range(XH):
                    nc.tensor.matmul(
                        ps[:, n, :], w_t[:, k, m * 128:(m + 1) * 128],
                        src[:, k, n * 512:(n + 1) * 512],
                        start=(k == 0), stop=(k == KB - 1))
            bal.copy(dst[:, m, :], ps[:], 1024)

        def emit_transpose(b, yt):
            s = st[b]
            ktp = pc.tile([128, 4, 128], F16, tag="c", name=f"ktp{b}_{yt}")
            for k in range(KB):
                nc.tensor.transpose(
                    ktp[:, k, :], s["K"][:, k, yt * 128:(yt + 1) * 128],
                    ident[:])
            bal.copy(s["KT"][:, yt, :], ktp[:], 512)

        def emit_a(b, i, yt, lat_ok=False):
            s = st[b]
            hp, xh = i // 2, i % 2
            xs = slice(xh * 512, (xh + 1) * 512)
            A = pa.tile([128, 2, 512], F32, tag="a", name=f"a_{b}_{i}_{yt}")
            for j in range(2):
                hs = slice(64 * j, 64 * (j + 1))
                nc.tensor.matmul(
                    A[:, j, :], s["K"][hs, hp, yt * 128:(yt + 1) * 128],
                    s["Q"][hs, hp, xs], start=True, stop=True)
            Am = amp.tile([128, 2, 512], F16, tag="am", bufs=18,
                          name=f"am_{b}_{i}_{yt}")
            mtf_b = s["MTI"][:, yt, xs].unsqueeze(1).broadcast_to((128, 2, 512))
            bal.relu_mask_pair(Am, A, mtf_b, s["MTI"][:, yt, xs], amp,
                               f"at_{b}_{i}_{yt}", lat_ok=lat_ok)
            s["ams"][(i, yt)] = Am

        def emit_c(b, i, yt):
            s = st[b]
            hp, xh = i // 2, i % 2
            # both heads accumulate into ONE bank: j0 at partitions 0-63
            # (col group 0), j1 at 64-127 (col group 64).
            if yt == 0:
                s["cs"][i] = pc.tile([128, 512], F32, tag="c", name=f"c_{b}_{i}")
            C = s["cs"][i]
            for j in range(2):
                nc.tensor.matmul(
                    C[64 * j:64 * (j + 1), :],
                    s["KT"][:, yt, hp * 128 + 64 * j: hp * 128 + 64 * (j + 1)],
                    s["ams"][(i, yt)][:, j, :],
                    start=(yt == 0), stop=(yt == YT - 1),
                    skip_group_check=True)
            if yt == YT - 1:
                # E = Q + C (C is pre-normalized via MTI); DVE only
                # (ACT has no tensor-tensor, GpSimd has no PSUM access)
                xs = slice(xh * 512, (xh + 1) * 512)
                nc.vector.tensor_tensor(s["E"][:, hp, xs], C[:],
                                        s["Q"][:, hp, xs], AL.add)
                bal.t["v"] += bal.dve_psum(512)

        def emit_out(b, m):
            # n-outer: each x-half's accumulation group closes after 4
            # matmuls, so its copy + store DMA overlap the other half's
            # matmuls (shortens the kernel tail)
            s = st[b]
            ps = pa.tile([128, 2, 512], F32, tag="a", name=f"po{b}_{m}")
            oS = osp.tile([128, 2, 512], F32, tag="os", name=f"os{b}_{m}")
            for n in range(XH):
                for k in range(KB):
                    nc.tensor.matmul(ps[:, n, :],
                                     WOT[:, k, m * 128:(m + 1) * 128],
                                     s["E"][:, k, n * 512:(n + 1) * 512],
                                     start=(k == 0), stop=(k == KB - 1))
                bal.copy(oS[:, n, :], ps[:, n, :], 512)
                nc.sync.dma_start(
                    o_e.ap()[b, m * 128:(m + 1) * 128, n * 512:(n + 1) * 512],
                    oS[:, n, :])

        def junk(n):
            # identity transposes, discarded: keep the PE "busy" through
            # DMA-bound stretches so the HAM clock gate stays at 2.4 GHz.
            # All junk() calls precede the first emit_transpose in emission
            # order, so the pc-pool slot handoff from jk to the ktp tiles
            # is cleanly ordered.
            for i in range(n):
                nc.tensor.transpose(jk[:, i % 4, :], ident[:], ident[:])

        def fill(b, warm):
            """Projections + a-blocks 0/1 + KT transposes for batch b.
            warm: sprinkle junk transposes (batch 0 only, DMA-bound)."""
            J = (lambda n: junk(n)) if warm else (lambda n: None)
            emit_proj(b, WQT, "X", "Q", 0)
            J(10)
            emit_proj(b, WKT, "Y", "K", 0)
            J(10)
            emit_a(b, 0, 0, True); emit_a(b, 0, 1, True)
            emit_proj(b, WKT, "Y", "K", 1)
            J(8)
            emit_a(b, 0, 2, True); emit_a(b, 0, 3, True)
            emit_proj(b, WKT, "Y", "K", 2)
            J(8)
            emit_a(b, 0, 4, True); emit_a(b, 0, 5, True)
            emit_proj(b, WKT, "Y", "K", 3)
            J(8)
            emit_a(b, 0, 6, True); emit_a(b, 0, 7, True)
            emit_proj(b, WQT, "X", "Q", 1)
            J(8)
            for yt in range(YT):
                emit_a(b, 1, yt, lat_ok=True)
                emit_transpose(b, yt)
            if b + 1 < BPC:
                loaded[b + 1] = issue_loads(b + 1)

        def steady(b):
            # A(i) and C(i-2) interleaved per y-tile: the relu+mask demand
            # on DVE/ACT stays smooth (1 op per ~1.4us of PE work) and each
            # C matmul trails its Am op by two full blocks
            for i in range(2, 2 * HP):
                if i == 2:
                    emit_proj(b, WQT, "X", "Q", 2)
                if i == 4:
                    emit_proj(b, WQT, "X", "Q", 3)
                for yt in range(YT):
                    emit_a(b, i, yt)
                    emit_c(b, i - 2, yt)

        # ---- the woven whole-kernel schedule ----
        # Per batch: fill (projections + a-blocks 0,1 + transposes), steady
        # (a2..a7 against c0..c5), then the c6/c7 drain and the output
        # projection are INTERLEAVED with the next batch's fill so the
        # PSUM->SBUF copy and relu+mask load on DVE/ACT stays smooth across
        # the batch boundary (a solid block of those stalls the PE long
        # enough for the HAM clock gate to re-throttle it to 1.2 GHz).
        init_batch(0)
        fill(0, warm=True)
        steady(0)
        for b in range(1, BPC):
            p = b - 1
            init_batch(b)
            for yt in range(YT):
                emit_c(p, 2 * HP - 2, yt)
                if yt == 1:
                    emit_proj(b, WQT, "X", "Q", 0)
                if yt == 5:
                    emit_proj(b, WKT, "Y", "K", 0)
            for yt in range(YT):
                emit_c(p, 2 * HP - 1, yt)
                if yt < 4:
                    emit_a(b, 0, yt)
            emit_out(p, 0)
            emit_a(b, 0, 4, True); emit_a(b, 0, 5, True)
            emit_proj(b, WKT, "Y", "K", 1)
            emit_out(p, 1)
            emit_a(b, 0, 6, True); emit_a(b, 0, 7, True)
            emit_proj(b, WKT, "Y", "K", 2)
            emit_out(p, 2)
            emit_a(b, 1, 0); emit_a(b, 1, 1)
            emit_proj(b, WKT, "Y", "K", 3)
            emit_out(p, 3)
            emit_proj(b, WQT, "X", "Q", 1)
            for yt in range(YT):
                if yt >= 2:
                    emit_a(b, 1, yt)
                emit_transpose(b, yt)
            if b + 1 < BPC:
                loaded[b + 1] = issue_loads(b + 1)
            steady(b)
        # ---- last-batch drain + output projection, k-pipelined ----
        # E's k-blocks 0..2 (head-pairs 0..2) are complete before the c6/c7
        # drain, so the out-projection's k=0..2 accumulation matmuls weave
        # into the drain; only the k=3 matmuls, copies and store DMAs remain
        # after the final E-add, cutting the kernel tail from ~13.6us to ~4.
        last = BPC - 1
        ops = {}

        def out_partial(m, k):
            if m not in ops:
                ops[m] = (pa.tile([128, 2, 512], F32, tag="a",
                                  name=f"po{last}_{m}"),
                          osp.tile([128, 2, 512], F32, tag="os",
                                   name=f"os{last}_{m}"))
            ps, _ = ops[m]
            for n in range(XH):
                nc.tensor.matmul(ps[:, n, :],
                                 WOT[:, k, m * 128:(m + 1) * 128],
                                 st[last]["E"][:, k, n * 512:(n + 1) * 512],
                                 start=(k == 0), stop=(k == KB - 1))

        chunks = [(m, k) for m in range(2) for k in range(KB - 1)]
        ci = 0
        for i in (2 * HP - 2, 2 * HP - 1):
            for yt in range(YT):
                emit_c(last, i, yt)
                if yt % 2 == 0 and ci < len(chunks):
                    out_partial(*chunks[ci])
                    ci += 1
        while ci < len(chunks):
            out_partial(*chunks[ci])
            ci += 1
        # m0/m1 close with one k=3 matmul each; m2/m3 run in full while the
        # m0/m1 copies and store DMAs drain on the other engines
        out_partial(0, KB - 1)
        out_partial(1, KB - 1)

        def out_store(m, n, eng=None):
            ps, oS = ops[m]
            if eng is None:
                bal.copy(oS[:, n, :], ps[:, n, :], 512)
            elif eng == "v":
                nc.vector.tensor_copy(oS[:, n, :], ps[:, n, :])
            else:
                nc.scalar.copy(oS[:, n, :], ps[:, n, :])
            nc.sync.dma_start(
                o_e.ap()[last, m * 128:(m + 1) * 128, n * 512:(n + 1) * 512],
                oS[:, n, :])

        out_store(0, 0)
        out_store(0, 1)
        for k in range(KB):
            out_partial(2, k)
        out_store(1, 0)
        out_store(1, 1)
        for k in range(KB):
            out_partial(3, k)
        out_store(2, 0, "v")
        out_store(2, 1, "s")
        out_store(3, 0, "v")
        out_store(3, 1, "s")


def _get_nc():
    if "nc" not in _CACHE:
        _CACHE["nc"] = _build()
    return _CACHE["nc"]


def kernel(x, y, xy_mask, wq, wk, wo):
    nc = _get_nc()
    xf = x.astype(np.float16)
    yf = y.astype(np.float16)
    # fold the attention scale and the per-row 1/nel normalization into the
    # transposed mask on the host: MTI[y, x] = mask[x, y] / (8 * max(nel_x, 1))
    nel = xy_mask.sum(axis=2, dtype=np.float32)           # (B, Lx)
    inv = 1.0 / (INV_SCALE * np.maximum(nel, 1.0))
    mtt = (xy_mask.transpose(0, 2, 1).astype(np.float32)
           * inv[:, None, :]).astype(np.float16)
    mtt = np.ascontiguousarray(mtt)
    w_all = np.stack([wq.T, wk.T, (0.5 * wo).T]).astype(np.float16)
    w_all = np.ascontiguousarray(w_all)
    in_maps = [
        {"x": xf[c * BPC:(c + 1) * BPC], "y": yf[c * BPC:(c + 1) * BPC],
         "mt": mtt[c * BPC:(c + 1) * BPC], "w_all": w_all}
        for c in range(N_CORES)
    ]
    res = run_bass_kernel_spmd(nc, in_maps, list(range(N_CORES)), trace=TRACE)
    if TRACE:
        _CACHE["last_exec_time_ns"] = res.exec_time_ns
        _CACHE["last_profile_json"] = res.profile_json
    return np.concatenate([res.results[c]["o"] for c in range(N_CORES)], axis=0)


# revision 39
# speedup vs baseline: 1.0111x; 1.0068x over previous
"""MultiHeadSimilarity kernel for 8 Trainium2 NeuronCores.

Reference computation (per batch b):
    Q = wq @ x[b];  K = wk @ y[b]                       (channel-mixing matmuls)
    per head h (d=64):  A = relu(Qh^T Kh) * scale, masked by xy_mask
    C = A @ Kh^T, normalized per-row by 1/max(sum(mask, y), 1)
    out = wo @ (0.5 * (Q + C))

Sharding: data-parallel over batch; 16 batches / 8 cores = 2 per core.
Weights replicated. No cross-core communication.

Device algorithm (per core, fp16 compute with fp32 PSUM accumulation):
  - The mask row-normalization (1/(8*max(nel,1)), which also folds the
    1/sqrt(d) attention scale) is computed on the HOST and pre-multiplied
    into the transposed mask -> MTI.  The device then computes
    Am = relu(A) * MTI and C comes out of the PE pre-normalized; E is a
    single add of C (PSUM) and Q.  This removes the ones^T@mask row-count
    matmuls, the reciprocal chain and the per-tile E multiply.
  - KT (the K transpose needed as the C-contraction stationary) is made by
    PE transpose instructions from K (4 per y-tile, via an identity), not by
    a second full projection: 4096 instead of 16384 PE columns per batch.
  - A is computed transposed (y on partitions) per head; relu+mask are
    greedily balanced between DVE (fused scalar_tensor_tensor) and ScalarE
    relu + DVE f16 multiply.  0.5 is folded into woT on the host.
  - The whole kernel is emitted as ONE woven instruction schedule: the PE
    queue is in-order, so A-matmuls run two blocks ahead of the C-matmuls
    that consume their relu+mask outputs, projections/KT-transposes weave
    into the early blocks, one batch's drain + output projection interleave
    with the next batch's fill, and the last batch's output projection
    pre-accumulates its k=0..2 terms during the final C drain.  Discarded
    identity transposes ("junk") pad the DMA-bound first ~25us so the HAM
    activity monitor holds the PE clock at 2.4 GHz instead of 1.2.
"""
import sys

if "/opt/trn_rl_repo" not in sys.path:
    sys.path.insert(0, "/opt/trn_rl_repo")

import numpy as np

import concourse.tile as tile
from concourse import bacc, masks, mybir
from concourse.bass_utils import run_bass_kernel_spmd

F16 = mybir.dt.float16
F32 = mybir.dt.float32
AL = mybir.AluOpType
RELU = mybir.ActivationFunctionType.Relu

N_CORES = 8
B, U, LX, LY, H, D = 16, 512, 1024, 1024, 8, 64
BPC = B // N_CORES          # batches per core
KB = U // 128               # 4  k-tiles over channels
HP = H // 2                 # 4  head pairs
YT = LY // 128              # 8  y tiles
XH = LX // 512              # 2  x halves
INV_SCALE = float(D) ** 0.5  # 8.0; attention scale = 1/8 (folded into MTI)

TRACE = False
_CACHE = {}


class Balance:
    """Greedy static load balancer between DVE and ACT.

    GpSimd is deliberately NOT used for element-wise work: its throughput is
    ~3x worse than modeled, its queue drowns in semaphore updates, and the
    resulting Am-supply stalls make the PE idle long enough for the HAM
    activity monitor to re-throttle the PE clock to 1.2 GHz (measured: 52%
    of the runtime at K=4/8, a net regression).
    """

    # measured per-op costs (ns) as functions of free-dim length
    @staticmethod
    def dve_psum(fd):          # psum-f32-source op (stt / cast / copy): 1x mode
        return (fd + 130) / 0.96

    @staticmethod
    def dve_f16(fd):           # f16 x f16 -> f16 tensor_tensor: 2x mode
        return (fd / 2 + 130) / 0.96

    @staticmethod
    def act_op(fd):            # scalar-engine activation / copy
        return (fd + 290) / 1.2

    GPS_TT = 2400.0            # gpsimd f16 tensor_tensor, per (128,2,512) tile

    def __init__(self, nc):
        self.nc = nc
        self.t = {"v": 0.0, "s": 0.0, "g": 0.0}

    def copy(self, dst, src, fd):
        """psum -> sbuf copy, DVE or ACT."""
        dve = self.dve_psum(fd)
        act = self.act_op(fd)
        if self.t["v"] + dve <= self.t["s"] + act:
            self.t["v"] += dve
            self.nc.vector.tensor_copy(dst, src)
        else:
            self.t["s"] += act
            self.nc.scalar.copy(dst, src)

    def relu_mask_pair(self, out, a_ps, mtf_b, mti_2d, tmp_pool, name,
                       lat_ok=False):
        """out[(128,2,512) f16] = relu(a_ps) * MTI broadcast over head dim.

        route 1: one fused DVE scalar_tensor_tensor at FD=1024;
        route 2: ScalarE relu at FD=1024 + DVE f16 2x multiply;
        route 3 (lat_ok only): ScalarE relu + GpSimd multiply.  GpSimd is
        ~3.5x slower than DVE and its op latency is ~2.5us, so it only gets
        tiles whose consumer is more than a block away (the fill phase,
        where DVE/ACT capacity is what limits Am inventory build-up) and
        only a handful per batch (a saturated Pool queue stalls everything).
        """
        stt = self.dve_psum(1024)
        act = self.act_op(1024)
        mul_v = self.dve_f16(1024)
        m1 = max(self.t["v"] + stt, self.t["s"], self.t["g"])
        m2 = max(self.t["v"] + mul_v, self.t["s"] + act, self.t["g"])
        # measured: even ~6 GpSimd multiplies per batch slow the whole run
        # (Pool-queue semaphore overhead + SBUF-port contention with DVE);
        # route 3 kept for reference but disabled
        gps_ok = False and lat_ok and self.t["g"] + self.GPS_TT <= min(
            self.t["v"], self.t["s"])
        m3 = max(self.t["v"], self.t["s"] + act, self.t["g"] + self.GPS_TT) \
            if gps_ok else float("inf")
        best = min(m1, m2, m3)
        if best == m1:
            self.t["v"] += stt
            self.nc.vector.scalar_tensor_tensor(out[:], a_ps[:], 0.0, mtf_b,
                                                AL.max, AL.mult)
        else:
            at = tmp_pool.tile([128, 2, 512], F16, tag="at", name=name)
            self.t["s"] += act
            self.nc.scalar.activation(at[:], a_ps[:], RELU)
            if best == m2:
                self.t["v"] += mul_v
                self.nc.vector.tensor_tensor(out[:], at[:], mtf_b, AL.mult)
            else:
                self.t["g"] += self.GPS_TT
                for j in range(2):
                    self.nc.gpsimd.tensor_tensor(out[:, j, :], at[:, j, :],
                                                 mti_2d, AL.mult)


def _build():
    nc = bacc.Bacc("TRN2", target_bir_lowering=False, debug=False,
                   num_devices=N_CORES)
    x_e = nc.dram_tensor("x", [BPC, U, LX], F16, kind="ExternalInput")
    y_e = nc.dram_tensor("y", [BPC, U, LY], F16, kind="ExternalInput")
    mt_e = nc.dram_tensor("mt", [BPC, LY, LX], F16, kind="ExternalInput")
    w_all_e = nc.dram_tensor("w_all", [3, U, U], F16, kind="ExternalInput")
    o_e = nc.dram_tensor("o", [BPC, U, LX], F32, kind="ExternalOutput")

    with tile.TileContext(nc) as tc:
        _emit(nc, tc, x_e, y_e, mt_e, w_all_e, o_e)
    nc.compile()
    return nc


def _emit(nc, tc, x_e, y_e, mt_e, w_all_e, o_e):
    import contextlib
    bal = Balance(nc)
    ctx = contextlib.ExitStack()
    with ctx:
        wp = ctx.enter_context(tc.tile_pool(name="wp", bufs=1))
        io = ctx.enter_context(tc.tile_pool(name="io", bufs=2))
        pr = ctx.enter_context(tc.tile_pool(name="pr", bufs=2))
        amp = ctx.enter_context(tc.tile_pool(name="amp", bufs=4))
        osp = ctx.enter_context(tc.tile_pool(name="osp", bufs=4))
        pa = ctx.enter_context(tc.tile_pool(name="pa", bufs=3, space="PSUM"))
        pc = ctx.enter_context(tc.tile_pool(name="pc", bufs=2, space="PSUM"))

        # weights, loaded once (per-k so the first projection can start early)
        WQT = wp.tile([128, KB, U], F16, tag="wqt")
        WKT = wp.tile([128, KB, U], F16, tag="wkt")
        WOT = wp.tile([128, KB, U], F16, tag="wot")
        for wi, w_t in enumerate((WQT, WKT)):
            for k in range(KB):
                nc.scalar.dma_start(
                    w_t[:, k, :], w_all_e.ap()[wi, k * 128:(k + 1) * 128, :])
        ident = wp.tile([128, 128], F16, tag="ident")
        masks.make_identity(nc, ident[:])

        def issue_loads(b):
            """DMA-issue one batch's inputs; returns the SBUF tiles.

            X fully precedes Y precedes MTI (the order the PE consumes
            them), and consecutive k/y tiles alternate between the sync and
            gpsimd queues so each tensor streams at the combined bandwidth
            instead of serializing behind one ring.
            """
            X = io.tile([128, KB, LX], F16, tag="x", name=f"x{b}")
            Y = io.tile([128, KB, LY], F16, tag="y", name=f"y{b}")
            for src, dst in ((x_e, X), (y_e, Y)):
                for k in range(KB):
                    (nc.sync if k % 2 == 0 else nc.gpsimd).dma_start(
                        dst[:, k, :], src.ap()[b, k * 128:(k + 1) * 128, :])
            MTI = io.tile([128, YT, LX], F16, tag="mti", name=f"mti{b}")
            # batch 0 only: the first two mask tiles ride the scalar queue
            # (free once the wq/wk chunks are in) — the fill's relu+mask ops
            # are otherwise mask-starved while MTI queues behind X and Y.
            # Later batches prefetch during compute, when the scalar queue
            # is busy with relu work.
            if b == 0:
                qs = [nc.scalar, nc.scalar, nc.sync, nc.gpsimd,
                      nc.sync, nc.gpsimd, nc.sync, nc.gpsimd]
            else:
                qs = [nc.sync, nc.gpsimd] * 4
            for t in range(YT):
                qs[t].dma_start(
                    MTI[:, t, :], mt_e.ap()[b, t * 128:(t + 1) * 128, :])
            return X, Y, MTI

        loaded = {0: issue_loads(0)}
        # Warm-up: ~30 back-to-back identity transposes (results discarded).
        # The HAM clock gate keeps the PE at 1.2 GHz until it sees ~3.4us of
        # sustained activity; without this burst the whole DMA-bound fill
        # phase (first ~25us of matmuls) runs at half clock.
        jk = pc.tile([128, 4, 128], F16, tag="c", name="warmup")
        for i in range(40):
            nc.tensor.transpose(jk[:, i % 4, :], ident[:], ident[:])
        # WOT is not needed until the first output projection (~80us in);
        # keeping it out of the startup window frees 0.5MB of HBM bandwidth
        # for the batch-0 activations the PE is waiting on
        for k in range(KB):
            nc.scalar.dma_start(
                WOT[:, k, :], w_all_e.ap()[2, k * 128:(k + 1) * 128, :])

        st = {}   # per-batch state

        def init_batch(b):
            X, Y, MTI = loaded.pop(b)
            st[b] = dict(
                X=X, Y=Y, MTI=MTI,
                Q=pr.tile([128, KB, LX], F16, tag="q", name=f"q{b}"),
                K=pr.tile([128, KB, LY], F16, tag="k", name=f"k{b}"),
                KT=pr.tile([128, YT, U], F16, tag="kt", name=f"kt{b}"),
                E=pr.tile([128, KB, LX], F16, tag="e", name=f"e{b}"),
                ams={}, cs={},
            )

        def emit_proj(b, w_t, sk, dk, m):
            s = st[b]
            src, dst = s[sk], s[dk]
            ps = pa.tile([128, 2, 512], F32, tag="a", name=f"pj{b}_{dk}_{m}")
            for k in range(KB):
                for n in range(XH):
                    nc.tensor.matmul(
                        ps[:, n, :], w_t[:, k, m * 128:(m + 1) * 128],
                        src[:, k, n * 512:(n + 1) * 512],
                        start=(k == 0), stop=(k == KB - 1))
            bal.copy(dst[:, m, :], ps[:], 1024)

        def emit_transpose(b, yt):
            s = st[b]
            ktp = pc.tile([128, 4, 128], F16, tag="c", name=f"ktp{b}_{yt}")
            for k in range(KB):
                nc.tensor.transpose(
                    ktp[:, k, :], s["K"][:, k, yt * 128:(yt + 1) * 128],
                    ident[:])
            bal.copy(s["KT"][:, yt, :], ktp[:], 512)

        def emit_a(b, i, yt, lat_ok=False):
            s = st[b]
            hp, xh = i // 2, i % 2
            xs = slice(xh * 512, (xh + 1) * 512)
            A = pa.tile([128, 2, 512], F32, tag="a", name=f"a_{b}_{i}_{yt}")
            for j in range(2):
                hs = slice(64 * j, 64 * (j + 1))
                nc.tensor.matmul(
                    A[:, j, :], s["K"][hs, hp, yt * 128:(yt + 1) * 128],
                    s["Q"][hs, hp, xs], start=True, stop=True)
            Am = amp.tile([128, 2, 512], F16, tag="am", bufs=18,
                          name=f"am_{b}_{i}_{yt}")
            mtf_b = s["MTI"][:, yt, xs].unsqueeze(1).broadcast_to((128, 2, 512))
            bal.relu_mask_pair(Am, A, mtf_b, s["MTI"][:, yt, xs], amp,
                               f"at_{b}_{i}_{yt}", lat_ok=lat_ok)
            s["ams"][(i, yt)] = Am

        def emit_c(b, i, yt):
            s = st[b]
            hp, xh = i // 2, i % 2
            # both heads accumulate into ONE bank: j0 at partitions 0-63
            # (col group 0), j1 at 64-127 (col group 64).
            if yt == 0:
                s["cs"][i] = pc.tile([128, 512], F32, tag="c", name=f"c_{b}_{i}")
            C = s["cs"][i]
            for j in range(2):
                nc.tensor.matmul(
                    C[64 * j:64 * (j + 1), :],
                    s["KT"][:, yt, hp * 128 + 64 * j: hp * 128 + 64 * (j + 1)],
                    s["ams"][(i, yt)][:, j, :],
                    start=(yt == 0), stop=(yt == YT - 1),
                    skip_group_check=True)
            if yt == YT - 1:
                # E = Q + C (C is pre-normalized via MTI); DVE only
                # (ACT has no tensor-tensor, GpSimd has no PSUM access)
                xs = slice(xh * 512, (xh + 1) * 512)
                nc.vector.tensor_tensor(s["E"][:, hp, xs], C[:],
                                        s["Q"][:, hp, xs], AL.add)
                bal.t["v"] += bal.dve_psum(512)

        def emit_out(b, m):
            # n-outer: each x-half's accumulation group closes after 4
            # matmuls, so its copy + store DMA overlap the other half's
            # matmuls (shortens the kernel tail)
            s = st[b]
            ps = pa.tile([128, 2, 512], F32, tag="a", name=f"po{b}_{m}")
            oS = osp.tile([128, 2, 512], F32, tag="os", name=f"os{b}_{m}")
            for n in range(XH):
                for k in range(KB):
                    nc.tensor.matmul(ps[:, n, :],
                                     WOT[:, k, m * 128:(m + 1) * 128],
                                     s["E"][:, k, n * 512:(n + 1) * 512],
                                     start=(k == 0), stop=(k == KB - 1))
                bal.copy(oS[:, n, :], ps[:, n, :], 512)
                nc.sync.dma_start(
                    o_e.ap()[b, m * 128:(m + 1) * 128, n * 512:(n + 1) * 512],
                    oS[:, n, :])

        def junk(n):
            # identity transposes, discarded: keep the PE "busy" through
            # DMA-bound stretches so the HAM clock gate stays at 2.4 GHz.
            # All junk() calls precede the first emit_transpose in emission
            # order, so the pc-pool slot handoff from jk to the ktp tiles
            # is cleanly ordered.
            for i in range(n):
                nc.tensor.transpose(jk[:, i % 4, :], ident[:], ident[:])

        def fill(b, warm):
            """Projections + a-blocks 0/1 + KT transposes for batch b.
            warm: sprinkle junk transposes (batch 0 only, DMA-bound)."""
            J = (lambda n: junk(n)) if warm else (lambda n: None)
            emit_proj(b, WQT, "X", "Q", 0)
            J(10)
            emit_proj(b, WKT, "Y", "K", 0)
            J(10)
            emit_a(b, 0, 0, True); emit_a(b, 0, 1, True)
            emit_proj(b, WKT, "Y", "K", 1)
            J(8)
            emit_a(b, 0, 2, True); emit_a(b, 0, 3, True)
            emit_proj(b, WKT, "Y", "K", 2)
            J(8)
            emit_a(b, 0, 4, True); emit_a(b, 0, 5, True)
            emit_proj(b, WKT, "Y", "K", 3)
            J(8)
            emit_a(b, 0, 6, True); emit_a(b, 0, 7, True)
            emit_proj(b, WQT, "X", "Q", 1)
            J(8)
            for yt in range(YT):
                emit_a(b, 1, yt, lat_ok=True)
                emit_transpose(b, yt)
            if b + 1 < BPC:
                loaded[b + 1] = issue_loads(b + 1)

        def steady(b):
            # A(i) and C(i-2) interleaved per y-tile: the relu+mask demand
            # on DVE/ACT stays smooth (1 op per ~1.4us of PE work) and each
            # C matmul trails its Am op by two full blocks
            for i in range(2, 2 * HP):
                if i == 2:
                    emit_proj(b, WQT, "X", "Q", 2)
                if i == 4:
                    emit_proj(b, WQT, "X", "Q", 3)
                for yt in range(YT):
                    emit_a(b, i, yt)
                    emit_c(b, i - 2, yt)

        # ---- the woven whole-kernel schedule ----
        # Per batch: fill (projections + a-blocks 0,1 + transposes), steady
        # (a2..a7 against c0..c5), then the c6/c7 drain and the output
        # projection are INTERLEAVED with the next batch's fill so the
        # PSUM->SBUF copy and relu+mask load on DVE/ACT stays smooth across
        # the batch boundary (a solid block of those stalls the PE long
        # enough for the HAM clock gate to re-throttle it to 1.2 GHz).
        init_batch(0)
        fill(0, warm=True)
        steady(0)
        for b in range(1, BPC):
            p = b - 1
            init_batch(b)
            for yt in range(YT):
                emit_c(p, 2 * HP - 2, yt)
                if yt == 1:
                    emit_proj(b, WQT, "X", "Q", 0)
                if yt == 5:
                    emit_proj(b, WKT, "Y", "K", 0)
            for yt in range(YT):
                emit_c(p, 2 * HP - 1, yt)
                if yt < 4:
                    emit_a(b, 0, yt)
            emit_out(p, 0)
            emit_a(b, 0, 4, True); emit_a(b, 0, 5, True)
            emit_proj(b, WKT, "Y", "K", 1)
            emit_out(p, 1)
            emit_a(b, 0, 6, True); emit_a(b, 0, 7, True)
            emit_proj(b, WKT, "Y", "K", 2)
            emit_out(p, 2)
            emit_a(b, 1, 0); emit_a(b, 1, 1)
            emit_proj(b, WKT, "Y", "K", 3)
            emit_out(p, 3)
            emit_proj(b, WQT, "X", "Q", 1)
            for yt in range(YT):
                if yt >= 2:
                    emit_a(b, 1, yt)
                emit_transpose(b, yt)
            if b + 1 < BPC:
                loaded[b + 1] = issue_loads(b + 1)
            steady(b)
        # ---- last-batch drain + output projection, k-pipelined ----
        # E's k-blocks 0..2 (head-pairs 0..2) are complete before the c6/c7
        # drain, so the out-projection's k=0..2 accumulation matmuls weave
        # into the drain; only the k=3 matmuls, copies and store DMAs remain
        # after the final E-add, cutting the kernel tail from ~13.6us to ~4.
        last = BPC - 1
        ops = {}

        def out_partial(m, k):
            if m not in ops:
                ops[m] = (pa.tile([128, 2, 512], F32, tag="a",
                                  name=f"po{last}_{m}"),
                          osp.tile([128, 2, 512], F32, tag="os",
                                   name=f"os{last}_{m}"))
            ps, _ = ops[m]
            for n in range(XH):
                nc.tensor.matmul(ps[:, n, :],
                                 WOT[:, k, m * 128:(m + 1) * 128],
                                 st[last]["E"][:, k, n * 512:(n + 1) * 512],
                                 start=(k == 0), stop=(k == KB - 1))

        chunks = [(m, k) for m in range(2) for k in range(KB - 1)]
        ci = 0
        for i in (2 * HP - 2, 2 * HP - 1):
            for yt in range(YT):
                emit_c(last, i, yt)
                if yt % 2 == 0 and ci < len(chunks):
                    out_partial(*chunks[ci])
                    ci += 1
        while ci < len(chunks):
            out_partial(*chunks[ci])
            ci += 1
        # m0/m1 close with one k=3 matmul each; m2/m3 run in full while the
        # m0/m1 copies and store DMAs drain on the other engines
        out_partial(0, KB - 1)
        out_partial(1, KB - 1)

        def out_store(m, n, eng=None):
            ps, oS = ops[m]
            if eng is None:
                bal.copy(oS[:, n, :], ps[:, n, :], 512)
            elif eng == "v":
                nc.vector.tensor_copy(oS[:, n, :], ps[:, n, :])
            else:
                nc.scalar.copy(oS[:, n, :], ps[:, n, :])
            nc.sync.dma_start(
                o_e.ap()[last, m * 128:(m + 1) * 128, n * 512:(n + 1) * 512],
                oS[:, n, :])

        out_store(0, 0)
        out_store(0, 1)
        for k in range(KB):
            out_partial(2, k)
        out_store(1, 0)
        out_store(1, 1)
        for k in range(KB):
            out_partial(3, k)
        out_store(2, 0, "v")
        out_store(2, 1, "s")
        out_store(3, 0, "v")
        out_store(3, 1, "s")


def _get_nc():
    if "nc" not in _CACHE:
        _CACHE["nc"] = _build()
    return _CACHE["nc"]


def kernel(x, y, xy_mask, wq, wk, wo):
    nc = _get_nc()
    xf = x.astype(np.float16)
    yf = y.astype(np.float16)
    # fold the attention scale and the per-row 1/nel normalization into the
    # transposed mask on the host: MTI[y, x] = mask[x, y] / (8 * max(nel_x, 1))
    nel = xy_mask.sum(axis=2, dtype=np.float32)           # (B, Lx)
    inv = 1.0 / (INV_SCALE * np.maximum(nel, 1.0))
    mtt = (xy_mask.transpose(0, 2, 1).astype(np.float32)
           * inv[:, None, :]).astype(np.float16)
    mtt = np.ascontiguousarray(mtt)
    w_all = np.stack([wq.T, wk.T, (0.5 * wo).T]).astype(np.float16)
    w_all = np.ascontiguousarray(w_all)
    in_maps = [
        {"x": xf[c * BPC:(c + 1) * BPC], "y": yf[c * BPC:(c + 1) * BPC],
         "mt": mtt[c * BPC:(c + 1) * BPC], "w_all": w_all}
        for c in range(N_CORES)
    ]
    res = run_bass_kernel_spmd(nc, in_maps, list(range(N_CORES)), trace=TRACE)
    if TRACE:
        _CACHE["last_exec_time_ns"] = res.exec_time_ns
        _CACHE["last_profile_json"] = res.profile_json
    return np.concatenate([res.results[c]["o"] for c in range(N_CORES)], axis=0)
